# revision 6
# baseline (speedup 1.0000x reference)
"""Trainium2 Bass kernel for the BRBboxHead problem.

Computes, for fused_feats [32, 256, 4096]:
    h  = relu(BN0(W0 @ x))          (1x1 conv + BN folded on host)
    h  = relu(BN1(W1 @ h))
    cls = Wc @ h + bc               -> sem_scores [32, 4096, 18]
    reg = Wr @ h + br
    refined_angle    = coarse_angle + reg[0]
    refined_distance = coarse_distance + reg[1:7]

Sharding: data-parallel over batch, 4 batches per core across 8 cores.
Weights are folded host-side (BN scale/shift into W/b) and replicated.

Device pipeline per 512-position tile (channel-major trunk):
    DMA x [2x128, 512] -> PE fp32 matmuls (K=256 via 2-step PSUM accum)
    -> ACT relu+bias -> PE matmul (K=128) -> ACT relu+bias
    -> PE heads matmul [128,32]^T @ h1 -> [32, 512] PSUM
    -> ACT identity+bias(bc|br|0) -> DVE 32x32 stream transpose
    -> DVE add of coarse residuals on reg columns -> strided DMA out.
"""
import os
import sys

sys.path.insert(0, '/opt/trn_rl_repo')

import numpy as np

import concourse.bass as bass
import concourse.mybir as mybir
import concourse.tile as tile
from concourse.bass_utils import run_bass_kernel_spmd

# Problem shapes (hardcoded per contest contract).
B, CIN, N = 32, 256, 4096
C1 = 128
NCLS, NREG = 18, 7
CH = 32               # head channels padded 25 -> 32 for the stream transpose
N_CORES = 8
BPC = B // N_CORES    # batches per core
NT = 512              # positions per tile
BN_EPS = 1e-5

F32 = mybir.dt.float32
# Matmul dtype: float32r streams at full PE rate (vs 4 cyc/row for fp32).
_MM_DT = {
    "f32": mybir.dt.float32,
    "f32r": mybir.dt.float32r,
}[os.environ.get("BRB_MM_DT", "f32r")]


def _split_multi_waits(nc):
    """The walrus build here rejects instructions carrying more than one
    sync wait. Hoist all but the last wait of each instruction onto NOPs
    inserted just before it on the same engine — engines execute in
    order, so waiting on each sem in sequence is equivalent."""
    for f in nc.m.functions:
        for bb in f.blocks:
            out = []
            changed = False
            for inst in list(bb.instructions):
                si = inst.sync_info
                if si is not None and len(si.on_wait) > 1:
                    waits = list(si.on_wait)
                    for w in waits[:-1]:
                        nop = mybir.InstNoOp(name=nc.get_next_instruction_name())
                        nop.engine = inst.engine
                        nop.sync_info = mybir.SyncInfo(on_wait=[w], on_update=[])
                        out.append(nop)
                    inst.sync_info = mybir.SyncInfo(
                        on_wait=[waits[-1]], on_update=list(si.on_update)
                    )
                    changed = True
                out.append(inst)
            if changed:
                bb.instructions = out


def _build_program():
    nc = bass.Bass("TRN2", target_bir_lowering=False, debug=False)

    x = nc.dram_tensor("x", [BPC, CIN, N], F32, kind="ExternalInput").ap()
    cc = nc.dram_tensor("cc", [BPC, N, NREG], F32, kind="ExternalInput").ap()
    w0a = nc.dram_tensor("w0a", [128, C1], F32, kind="ExternalInput").ap()
    w0b = nc.dram_tensor("w0b", [128, C1], F32, kind="ExternalInput").ap()
    w1 = nc.dram_tensor("w1", [C1, C1], F32, kind="ExternalInput").ap()
    wh = nc.dram_tensor("wh", [C1, CH], F32, kind="ExternalInput").ap()
    b0 = nc.dram_tensor("b0", [C1, 1], F32, kind="ExternalInput").ap()
    b1 = nc.dram_tensor("b1", [C1, 1], F32, kind="ExternalInput").ap()
    bh = nc.dram_tensor("bh", [CH, 1], F32, kind="ExternalInput").ap()
    sem_o = nc.dram_tensor("sem", [BPC, N, NCLS], F32, kind="ExternalOutput").ap()
    reg_o = nc.dram_tensor("reg", [BPC, N, NREG], F32, kind="ExternalOutput").ap()

    relu = mybir.ActivationFunctionType.Relu
    ident = mybir.ActivationFunctionType.Identity

    with tile.TileContext(nc) as tc:
        with (
            tc.tile_pool(name="consts", bufs=1) as consts,
            tc.tile_pool(name="xin", bufs=6) as xpool,
            tc.tile_pool(name="hmid", bufs=3) as hpool,
            tc.tile_pool(name="tout", bufs=4) as tpool,
            tc.tile_pool(name="ccin", bufs=3) as ccpool,
            tc.tile_pool(name="ps_trunk", bufs=2, space="PSUM") as pstrunk,
            tc.tile_pool(name="ps_head", bufs=2, space="PSUM") as pshead,
        ):
            w0a_sb = consts.tile([128, C1], _MM_DT, tag="w0a")
            nc.sync.dma_start(w0a_sb[:], w0a[:].bitcast(_MM_DT))
            w0b_sb = consts.tile([128, C1], _MM_DT, tag="w0b")
            nc.sync.dma_start(w0b_sb[:], w0b[:].bitcast(_MM_DT))
            w1_sb = consts.tile([C1, C1], _MM_DT, tag="w1")
            nc.sync.dma_start(w1_sb[:], w1[:].bitcast(_MM_DT))
            wh_sb = consts.tile([C1, CH], _MM_DT, tag="wh")
            nc.sync.dma_start(wh_sb[:], wh[:].bitcast(_MM_DT))
            b0_sb = consts.tile([C1, 1], F32, tag="b0")
            nc.sync.dma_start(b0_sb[:], b0[:])
            b1_sb = consts.tile([C1, 1], F32, tag="b1")
            nc.sync.dma_start(b1_sb[:], b1[:])
            bh_sb = consts.tile([CH, 1], F32, tag="bh")
            nc.sync.dma_start(bh_sb[:], bh[:])

            for b in range(BPC):
                for t in range(N // NT):
                    n0 = t * NT
                    xa = xpool.tile([128, NT], _MM_DT, tag="xa")
                    nc.sync.dma_start(xa[:], x[b, 0:128, n0:n0 + NT].bitcast(_MM_DT))
                    xb = xpool.tile([128, NT], _MM_DT, tag="xb")
                    nc.sync.dma_start(xb[:], x[b, 128:256, n0:n0 + NT].bitcast(_MM_DT))
                    ccs = ccpool.tile([32, (NT // 32) * NREG], F32, tag="cc")
                    nc.sync.dma_start(
                        ccs[:].rearrange("p (k c) -> p k c", c=NREG),
                        cc[b, n0:n0 + NT, :].rearrange("(k p) c -> p k c", p=32),
                    )

                    p0 = pstrunk.tile([128, NT], F32, tag="p0")
                    nc.tensor.matmul(
                        p0[:], w0a_sb[:], xa[:], start=True, stop=False,
                    )
                    nc.tensor.matmul(
                        p0[:], w0b_sb[:], xb[:], start=False, stop=True,
                    )
                    h0 = hpool.tile([128, NT], _MM_DT, tag="h0")
                    nc.scalar.activation(h0[:], p0[:], relu, bias=b0_sb[:, 0:1])

                    p1 = pstrunk.tile([128, NT], F32, tag="p1")
                    nc.tensor.matmul(p1[:], w1_sb[:], h0[:])
                    h1 = hpool.tile([128, NT], _MM_DT, tag="h1")
                    nc.scalar.activation(h1[:], p1[:], relu, bias=b1_sb[:, 0:1])

                    ph = pshead.tile([CH, NT], F32, tag="ph")
                    nc.tensor.matmul(ph[:], wh_sb[:], h1[:])
                    hb = tpool.tile([CH, NT], F32, tag="hb")
                    nc.scalar.activation(hb[:], ph[:], ident, bias=bh_sb[:, 0:1])

                    # 32x32 block transpose: t32[p, 32k+c] = hb[c, 32k+p]
                    t32 = tpool.tile([CH, NT], F32, tag="t32")
                    nc.vector.transpose(t32[:], hb[:])

                    # reg columns += coarse residuals
                    treg = t32[:].rearrange("p (k c) -> p k c", c=32)[:, :, NCLS:25]
                    ccv = ccs[:].rearrange("p (k c) -> p k c", c=NREG)
                    nc.vector.tensor_add(treg, treg, ccv)

                    tsem = t32[:].rearrange("p (k c) -> p k c", c=32)[:, :, 0:NCLS]
                    sem_dst = sem_o[b, n0:n0 + NT, :].rearrange(
                        "(k p) c -> p k c", p=32
                    )
                    nc.sync.dma_start(sem_dst, tsem)
                    reg_dst = reg_o[b, n0:n0 + NT, :].rearrange(
                        "(k p) c -> p k c", p=32
                    )
                    nc.sync.dma_start(reg_dst, treg)
    _split_multi_waits(nc)
    return nc


_NC_CACHE = {}


def _get_program():
    key = str(_MM_DT)
    if key not in _NC_CACHE:
        _NC_CACHE[key] = _build_program()
    return _NC_CACHE[key]


def _host_fold(inputs):
    """Fold BN into conv weights/biases; build per-core input maps."""
    f = lambda k: np.asarray(inputs[k], dtype=np.float32)
    W0, b0, g0, be0, m0, v0 = (f(k) for k in ("W0", "b0", "g0", "be0", "m0", "v0"))
    W1, b1, g1, be1, m1, v1 = (f(k) for k in ("W1", "b1", "g1", "be1", "m1", "v1"))
    Wc, bc, Wr, br = (f(k) for k in ("Wc", "bc", "Wr", "br"))

    s0 = g0 / np.sqrt(v0 + BN_EPS)
    W0f = W0 * s0[:, None]
    b0f = (b0 - m0) * s0 + be0
    s1 = g1 / np.sqrt(v1 + BN_EPS)
    W1f = W1 * s1[:, None]
    b1f = (b1 - m1) * s1 + be1

    w0T = np.ascontiguousarray(W0f.T)            # [256, 128]
    wh = np.zeros((C1, CH), np.float32)          # [128, 32]
    wh[:, 0:NCLS] = Wc.T
    wh[:, NCLS:25] = Wr.T
    bh = np.zeros((CH, 1), np.float32)
    bh[0:NCLS, 0] = bc
    bh[NCLS:25, 0] = br

    fused = np.asarray(inputs["fused_feats"], dtype=np.float32)
    angle = np.asarray(inputs["coarse_angle"], dtype=np.float32)
    dist = np.asarray(inputs["coarse_distance"], dtype=np.float32)
    cc = np.empty((B, N, NREG), np.float32)
    cc[:, :, 0] = angle
    cc[:, :, 1:] = dist

    shared = {
        "w0a": np.ascontiguousarray(w0T[0:128]),
        "w0b": np.ascontiguousarray(w0T[128:256]),
        "w1": np.ascontiguousarray(W1f.T),
        "wh": wh,
        "b0": b0f.reshape(C1, 1),
        "b1": b1f.reshape(C1, 1),
        "bh": bh,
    }
    in_maps = []
    for c in range(N_CORES):
        lo, hi = c * BPC, (c + 1) * BPC
        m = dict(shared)
        m["x"] = np.ascontiguousarray(fused[lo:hi])
        m["cc"] = np.ascontiguousarray(cc[lo:hi])
        in_maps.append(m)
    return in_maps


def _run(inputs, trace=False):
    nc = _get_program()
    in_maps = _host_fold(inputs)
    res = run_bass_kernel_spmd(
        nc, in_maps, core_ids=list(range(N_CORES)), trace=trace
    )
    sem = np.concatenate([res.results[c]["sem"] for c in range(N_CORES)], axis=0)
    reg = np.concatenate([res.results[c]["reg"] for c in range(N_CORES)], axis=0)
    sem_scores = np.ascontiguousarray(sem)
    refined_angle = np.ascontiguousarray(reg[:, :, 0])
    refined_distance = np.ascontiguousarray(reg[:, :, 1:])
    return (sem_scores, refined_angle, refined_distance), res


def kernel(**inputs):
    outs, _ = _run(inputs, trace=False)
    return outs


# revision 8
# speedup vs baseline: 1.3229x; 1.3229x over previous
"""Trainium2 Bass kernel for the BRBboxHead problem.

Computes, for fused_feats [32, 256, 4096]:
    h  = relu(BN0(W0 @ x))          (1x1 conv + BN folded on host)
    h  = relu(BN1(W1 @ h))
    cls = Wc @ h + bc               -> sem_scores [32, 4096, 18]
    reg = Wr @ h + br
    refined_angle    = coarse_angle + reg[0]
    refined_distance = coarse_distance + reg[1:7]

Sharding: data-parallel over batch, 4 batches per core across 8 cores.
Weights are folded host-side (BN scale/shift into W/b) and replicated.

Device pipeline per 512-position tile (channel-major trunk):
    DMA x [128, 2x512] -> PE matmuls (K=256 via 2-step PSUM accum)
    -> ACT relu+bias -> PE matmul (K=128) -> ACT relu+bias
    -> PE heads matmul [128,32]^T @ h1 -> [32, 512] PSUM
    -> ACT identity+bias(bc|br|0) -> DVE 32x32 stream transpose
    -> DVE add of coarse residuals on reg columns -> contiguous DMA out.

All DRAM-side layouts are chosen so every DMA is a contiguous (or
2KB-row) dump; the host does the cheap permutes in numpy.
"""
import os
import sys

sys.path.insert(0, '/opt/trn_rl_repo')

import numpy as np

import concourse.bass as bass
import concourse.mybir as mybir
import concourse.tile as tile
from concourse.bass_utils import run_bass_kernel_spmd

# Problem shapes (hardcoded per contest contract).
B, CIN, N = 32, 256, 4096
C1 = 128
NCLS, NREG = 18, 7
CH = 32               # head channels padded 25 -> 32 for the stream transpose
N_CORES = 8
BPC = B // N_CORES    # batches per core
NT = 512              # positions per tile
NTILES = N // NT
BN_EPS = 1e-5

F32 = mybir.dt.float32
# Matmul dtype: float32r streams at full PE rate (vs 4 cyc/row for fp32).
_MM_DT = {
    "f32": mybir.dt.float32,
    "f32r": mybir.dt.float32r,
}[os.environ.get("BRB_MM_DT", "f32r")]


def _split_multi_waits(nc):
    """The walrus build here rejects instructions carrying more than one
    sync wait. Hoist all but the last wait of each instruction onto NOPs
    inserted just before it on the same engine — engines execute in
    order, so waiting on each sem in sequence is equivalent."""
    for f in nc.m.functions:
        for bb in f.blocks:
            out = []
            changed = False
            for inst in list(bb.instructions):
                si = inst.sync_info
                if si is not None and len(si.on_wait) > 1:
                    waits = list(si.on_wait)
                    for w in waits[:-1]:
                        nop = mybir.InstNoOp(name=nc.get_next_instruction_name())
                        nop.engine = inst.engine
                        nop.sync_info = mybir.SyncInfo(on_wait=[w], on_update=[])
                        out.append(nop)
                    inst.sync_info = mybir.SyncInfo(
                        on_wait=[waits[-1]], on_update=list(si.on_update)
                    )
                    changed = True
                out.append(inst)
            if changed:
                bb.instructions = out


def _build_program():
    nc = bass.Bass("TRN2", target_bir_lowering=False, debug=False)

    # x laid out [BPC, 2, 128, N]: K-chunk-major so one DMA per tile
    x = nc.dram_tensor("x", [BPC, 2, 128, N], F32, kind="ExternalInput").ap()
    # coarse residuals pre-arranged to the post-transpose SBUF layout
    cc = nc.dram_tensor(
        "cc", [BPC, NTILES, 32, (NT // 32) * NREG], F32, kind="ExternalInput"
    ).ap()
    w0a = nc.dram_tensor("w0a", [128, C1], F32, kind="ExternalInput").ap()
    w0b = nc.dram_tensor("w0b", [128, C1], F32, kind="ExternalInput").ap()
    w1 = nc.dram_tensor("w1", [C1, C1], F32, kind="ExternalInput").ap()
    wh = nc.dram_tensor("wh", [C1, CH], F32, kind="ExternalInput").ap()
    b0 = nc.dram_tensor("b0", [C1, 1], F32, kind="ExternalInput").ap()
    b1 = nc.dram_tensor("b1", [C1, 1], F32, kind="ExternalInput").ap()
    bh = nc.dram_tensor("bh", [CH, 1], F32, kind="ExternalInput").ap()
    # raw transposed head tiles; host unpacks to sem/angle/distance
    out_o = nc.dram_tensor(
        "out", [BPC, NTILES, CH, NT], F32, kind="ExternalOutput"
    ).ap()

    relu = mybir.ActivationFunctionType.Relu
    ident = mybir.ActivationFunctionType.Identity

    with tile.TileContext(nc) as tc:
        with (
            tc.tile_pool(name="consts", bufs=1) as consts,
            tc.tile_pool(name="xin", bufs=3) as xpool,
            tc.tile_pool(name="hmid", bufs=3) as hpool,
            tc.tile_pool(name="tout", bufs=4) as tpool,
            tc.tile_pool(name="ccin", bufs=3) as ccpool,
            tc.tile_pool(name="ps_trunk", bufs=2, space="PSUM") as pstrunk,
            tc.tile_pool(name="ps_head", bufs=2, space="PSUM") as pshead,
        ):
            w0a_sb = consts.tile([128, C1], _MM_DT, tag="w0a")
            nc.sync.dma_start(w0a_sb[:], w0a[:].bitcast(_MM_DT))
            w0b_sb = consts.tile([128, C1], _MM_DT, tag="w0b")
            nc.sync.dma_start(w0b_sb[:], w0b[:].bitcast(_MM_DT))
            w1_sb = consts.tile([C1, C1], _MM_DT, tag="w1")
            nc.sync.dma_start(w1_sb[:], w1[:].bitcast(_MM_DT))
            wh_sb = consts.tile([C1, CH], _MM_DT, tag="wh")
            nc.sync.dma_start(wh_sb[:], wh[:].bitcast(_MM_DT))
            b0_sb = consts.tile([C1, 1], F32, tag="b0")
            nc.sync.dma_start(b0_sb[:], b0[:])
            b1_sb = consts.tile([C1, 1], F32, tag="b1")
            nc.sync.dma_start(b1_sb[:], b1[:])
            bh_sb = consts.tile([CH, 1], F32, tag="bh")
            nc.sync.dma_start(bh_sb[:], bh[:])

            for b in range(BPC):
                for t in range(NTILES):
                    n0 = t * NT
                    # one DMA per tile: the two K-chunks side by side
                    xt = xpool.tile([128, 2 * NT], _MM_DT, tag="xt")
                    nc.sync.dma_start(
                        xt[:].rearrange("p (j n) -> p j n", n=NT),
                        x[b, :, :, n0:n0 + NT].rearrange(
                            "j p n -> p j n"
                        ).bitcast(_MM_DT),
                    )
                    ccs = ccpool.tile([32, (NT // 32) * NREG], F32, tag="cc")
                    nc.sync.dma_start(ccs[:], cc[b, t])

                    p0 = pstrunk.tile([128, NT], F32, tag="p0")
                    nc.tensor.matmul(
                        p0[:], w0a_sb[:], xt[:, 0:NT], start=True, stop=False,
                    )
                    nc.tensor.matmul(
                        p0[:], w0b_sb[:], xt[:, NT:2 * NT], start=False, stop=True,
                    )
                    h0 = hpool.tile([128, NT], _MM_DT, tag="h0")
                    nc.scalar.activation(h0[:], p0[:], relu, bias=b0_sb[:, 0:1])

                    p1 = pstrunk.tile([128, NT], F32, tag="p1")
                    nc.tensor.matmul(p1[:], w1_sb[:], h0[:])
                    h1 = hpool.tile([128, NT], _MM_DT, tag="h1")
                    nc.scalar.activation(h1[:], p1[:], relu, bias=b1_sb[:, 0:1])

                    ph = pshead.tile([CH, NT], F32, tag="ph")
                    nc.tensor.matmul(ph[:], wh_sb[:], h1[:])
                    hb = tpool.tile([CH, NT], F32, tag="hb")
                    nc.scalar.activation(hb[:], ph[:], ident, bias=bh_sb[:, 0:1])

                    # 32x32 block transpose: t32[p, 32k+c] = hb[c, 32k+p]
                    t32 = tpool.tile([CH, NT], F32, tag="t32")
                    nc.vector.transpose(t32[:], hb[:])

                    # reg columns += coarse residuals
                    treg = t32[:].rearrange("p (k c) -> p k c", c=32)[:, :, NCLS:25]
                    ccv = ccs[:].rearrange("p (k c) -> p k c", c=NREG)
                    nc.vector.tensor_add(treg, treg, ccv)

                    nc.sync.dma_start(out_o[b, t], t32[:])
    if os.environ.get("BRB_SKIP_SPLIT") != "1":
        _split_multi_waits(nc)
    return nc


_NC_CACHE = {}


def _get_program():
    key = str(_MM_DT)
    if key not in _NC_CACHE:
        _NC_CACHE[key] = _build_program()
    return _NC_CACHE[key]


def _host_fold(inputs):
    """Fold BN into conv weights/biases; build per-core input maps."""
    f = lambda k: np.asarray(inputs[k], dtype=np.float32)
    W0, b0, g0, be0, m0, v0 = (f(k) for k in ("W0", "b0", "g0", "be0", "m0", "v0"))
    W1, b1, g1, be1, m1, v1 = (f(k) for k in ("W1", "b1", "g1", "be1", "m1", "v1"))
    Wc, bc, Wr, br = (f(k) for k in ("Wc", "bc", "Wr", "br"))

    s0 = g0 / np.sqrt(v0 + BN_EPS)
    W0f = W0 * s0[:, None]
    b0f = (b0 - m0) * s0 + be0
    s1 = g1 / np.sqrt(v1 + BN_EPS)
    W1f = W1 * s1[:, None]
    b1f = (b1 - m1) * s1 + be1

    w0T = np.ascontiguousarray(W0f.T)            # [256, 128]
    wh = np.zeros((C1, CH), np.float32)          # [128, 32]
    wh[:, 0:NCLS] = Wc.T
    wh[:, NCLS:25] = Wr.T
    bh = np.zeros((CH, 1), np.float32)
    bh[0:NCLS, 0] = bc
    bh[NCLS:25, 0] = br

    fused = np.asarray(inputs["fused_feats"], dtype=np.float32)
    # [B, 256, N] -> [B, 2, 128, N] K-chunk split (a reshape, no copy)
    fused = fused.reshape(B, 2, 128, N)

    angle = np.asarray(inputs["coarse_angle"], dtype=np.float32)
    dist = np.asarray(inputs["coarse_distance"], dtype=np.float32)
    cc = np.empty((B, N, NREG), np.float32)
    cc[:, :, 0] = angle
    cc[:, :, 1:] = dist
    # pre-arrange to the post-transpose SBUF layout:
    # cc_dev[b, t, p, k*7+c] = cc[b, t*512 + k*32 + p, c]
    cc_dev = np.ascontiguousarray(
        cc.reshape(B, NTILES, NT // 32, 32, NREG).transpose(0, 1, 3, 2, 4)
    ).reshape(B, NTILES, 32, (NT // 32) * NREG)

    shared = {
        "w0a": np.ascontiguousarray(w0T[0:128]),
        "w0b": np.ascontiguousarray(w0T[128:256]),
        "w1": np.ascontiguousarray(W1f.T),
        "wh": wh,
        "b0": b0f.reshape(C1, 1),
        "b1": b1f.reshape(C1, 1),
        "bh": bh,
    }
    in_maps = []
    for c in range(N_CORES):
        lo, hi = c * BPC, (c + 1) * BPC
        m = dict(shared)
        m["x"] = np.ascontiguousarray(fused[lo:hi])
        m["cc"] = np.ascontiguousarray(cc_dev[lo:hi])
        in_maps.append(m)
    return in_maps


def _run(inputs, trace=False):
    nc = _get_program()
    in_maps = _host_fold(inputs)
    res = run_bass_kernel_spmd(
        nc, in_maps, core_ids=list(range(N_CORES)), trace=trace
    )
    raw = np.concatenate([res.results[c]["out"] for c in range(N_CORES)], axis=0)
    # raw[b, t, p, 32k+c] = channel c at position t*512 + k*32 + p
    full = np.ascontiguousarray(
        raw.reshape(B, NTILES, 32, NT // 32, 32).transpose(0, 1, 3, 2, 4)
    ).reshape(B, N, 32)
    sem_scores = np.ascontiguousarray(full[:, :, 0:NCLS])
    refined_angle = np.ascontiguousarray(full[:, :, NCLS])
    refined_distance = np.ascontiguousarray(full[:, :, NCLS + 1:25])
    return (sem_scores, refined_angle, refined_distance), res


def kernel(**inputs):
    outs, _ = _run(inputs, trace=False)
    return outs


# revision 10
# speedup vs baseline: 2.1414x; 1.6187x over previous
"""Trainium2 Bass kernel for the BRBboxHead problem.

Computes, for fused_feats [32, 256, 4096]:
    h  = relu(BN0(W0 @ x))          (1x1 conv + BN folded on host)
    h  = relu(BN1(W1 @ h))
    cls = Wc @ h + bc               -> sem_scores [32, 4096, 18]
    reg = Wr @ h + br
    refined_angle    = coarse_angle + reg[0]
    refined_distance = coarse_distance + reg[1:7]

Sharding: data-parallel over batch, 4 batches per core across 8 cores.
Weights are folded host-side (BN scale/shift into W/b) and replicated.

Device pipeline per 512-position tile (channel-major trunk):
    DMA x [128, 2x512] -> PE matmuls (K=256 via 2-step PSUM accum)
    -> ACT relu+bias -> PE matmul (K=128) -> ACT relu+bias
    -> PE heads matmul [128,32]^T @ h1 -> [32, 512] PSUM
    -> ACT identity+bias(bc|br|0) -> DVE 32x32 stream transpose
    -> DVE add of coarse residuals on reg columns -> contiguous DMA out.

All DRAM-side layouts are chosen so every DMA is a contiguous (or
2KB-row) dump; the host does the cheap permutes in numpy.
"""
import os
import sys

sys.path.insert(0, '/opt/trn_rl_repo')

import numpy as np

import concourse.bass as bass
import concourse.mybir as mybir
import concourse.tile as tile
from concourse.bass_utils import run_bass_kernel_spmd

# Problem shapes (hardcoded per contest contract).
B, CIN, N = 32, 256, 4096
C1 = 128
NCLS, NREG = 18, 7
CH = 32               # head channels padded 25 -> 32 for the stream transpose
N_CORES = 8
BPC = B // N_CORES    # batches per core
NT = 512              # positions per tile
NTILES = N // NT
BN_EPS = 1e-5

F32 = mybir.dt.float32
# Matmul dtype: float32r streams at full PE rate (vs 4 cyc/row for fp32).
_MM_DT = {
    "f32": mybir.dt.float32,
    "f32r": mybir.dt.float32r,
}[os.environ.get("BRB_MM_DT", "f32r")]


def _split_multi_waits(nc):
    """The walrus build here rejects instructions carrying more than one
    sync wait. Hoist all but the last wait of each instruction onto NOPs
    inserted just before it on the same engine — engines execute in
    order, so waiting on each sem in sequence is equivalent."""
    for f in nc.m.functions:
        for bb in f.blocks:
            out = []
            changed = False
            for inst in list(bb.instructions):
                si = inst.sync_info
                if si is not None and len(si.on_wait) > 1:
                    waits = list(si.on_wait)
                    for w in waits[:-1]:
                        nop = mybir.InstNoOp(name=nc.get_next_instruction_name())
                        nop.engine = inst.engine
                        nop.sync_info = mybir.SyncInfo(on_wait=[w], on_update=[])
                        out.append(nop)
                    inst.sync_info = mybir.SyncInfo(
                        on_wait=[waits[-1]], on_update=list(si.on_update)
                    )
                    changed = True
                out.append(inst)
            if changed:
                bb.instructions = out


def _build_program():
    nc = bass.Bass("TRN2", target_bir_lowering=False, debug=False)

    # x laid out [BPC, 2, 128, N]: K-chunk-major so one DMA per tile
    x = nc.dram_tensor("x", [BPC, 2, 128, N], F32, kind="ExternalInput").ap()
    # coarse residuals pre-arranged to the post-transpose SBUF layout
    cc = nc.dram_tensor(
        "cc", [BPC, 32, NTILES * (NT // 32) * NREG], F32, kind="ExternalInput"
    ).ap()
    w0a = nc.dram_tensor("w0a", [128, C1], F32, kind="ExternalInput").ap()
    w0b = nc.dram_tensor("w0b", [128, C1], F32, kind="ExternalInput").ap()
    w1 = nc.dram_tensor("w1", [C1, C1], F32, kind="ExternalInput").ap()
    wh = nc.dram_tensor("wh", [C1, CH], F32, kind="ExternalInput").ap()
    b0 = nc.dram_tensor("b0", [C1, 1], F32, kind="ExternalInput").ap()
    b1 = nc.dram_tensor("b1", [C1, 1], F32, kind="ExternalInput").ap()
    bh = nc.dram_tensor("bh", [CH, 1], F32, kind="ExternalInput").ap()
    # raw transposed head tiles; host unpacks to sem/angle/distance
    out_o = nc.dram_tensor(
        "out", [BPC, CH, N], F32, kind="ExternalOutput"
    ).ap()

    relu = mybir.ActivationFunctionType.Relu
    ident = mybir.ActivationFunctionType.Identity

    with tile.TileContext(nc) as tc:
        with (
            tc.tile_pool(name="consts", bufs=1) as consts,
            tc.tile_pool(name="xin", bufs=2) as xpool,
            tc.tile_pool(name="hmid", bufs=3) as hpool,
            tc.tile_pool(name="tout", bufs=3) as tpool,
            tc.tile_pool(name="oout", bufs=2) as opool,
            tc.tile_pool(name="ccin", bufs=2) as ccpool,
            tc.tile_pool(name="ps_trunk", bufs=2, space="PSUM") as pstrunk,
            tc.tile_pool(name="ps_head", bufs=2, space="PSUM") as pshead,
        ):
            w0a_sb = consts.tile([128, C1], _MM_DT, tag="w0a")
            nc.sync.dma_start(w0a_sb[:], w0a[:].bitcast(_MM_DT))
            w0b_sb = consts.tile([128, C1], _MM_DT, tag="w0b")
            nc.sync.dma_start(w0b_sb[:], w0b[:].bitcast(_MM_DT))
            w1_sb = consts.tile([C1, C1], _MM_DT, tag="w1")
            nc.sync.dma_start(w1_sb[:], w1[:].bitcast(_MM_DT))
            wh_sb = consts.tile([C1, CH], _MM_DT, tag="wh")
            nc.sync.dma_start(wh_sb[:], wh[:].bitcast(_MM_DT))
            b0_sb = consts.tile([C1, 1], F32, tag="b0")
            nc.sync.dma_start(b0_sb[:], b0[:])
            b1_sb = consts.tile([C1, 1], F32, tag="b1")
            nc.sync.dma_start(b1_sb[:], b1[:])
            bh_sb = consts.tile([CH, 1], F32, tag="bh")
            nc.sync.dma_start(bh_sb[:], bh[:])

            CCW = (NT // 32) * NREG  # 112 cc columns per tile
            for b in range(BPC):
                # batch-granular DMAs: 16KB contiguous runs per partition
                xbig = xpool.tile([128, 2 * N], _MM_DT, tag="xt")
                nc.sync.dma_start(xbig[:, 0:N], x[b, 0].bitcast(_MM_DT))
                nc.sync.dma_start(xbig[:, N:2 * N], x[b, 1].bitcast(_MM_DT))
                ccb = ccpool.tile([32, NTILES * CCW], F32, tag="cc")
                nc.scalar.dma_start(ccb[:], cc[b])
                outb = opool.tile([CH, N], F32, tag="outb")

                for t in range(NTILES):
                    n0 = t * NT
                    p0 = pstrunk.tile([128, NT], F32, tag="p0")
                    nc.tensor.matmul(
                        p0[:], w0a_sb[:], xbig[:, n0:n0 + NT],
                        start=True, stop=False,
                    )
                    nc.tensor.matmul(
                        p0[:], w0b_sb[:], xbig[:, N + n0:N + n0 + NT],
                        start=False, stop=True,
                    )
                    h0 = hpool.tile([128, NT], _MM_DT, tag="h0")
                    nc.scalar.activation(h0[:], p0[:], relu, bias=b0_sb[:, 0:1])

                    p1 = pstrunk.tile([128, NT], F32, tag="p1")
                    nc.tensor.matmul(p1[:], w1_sb[:], h0[:])
                    h1 = hpool.tile([128, NT], _MM_DT, tag="h1")
                    nc.scalar.activation(h1[:], p1[:], relu, bias=b1_sb[:, 0:1])

                    ph = pshead.tile([CH, NT], F32, tag="ph")
                    nc.tensor.matmul(ph[:], wh_sb[:], h1[:])
                    hb = tpool.tile([CH, NT], F32, tag="hb")
                    nc.vector.tensor_scalar_add(hb[:], ph[:], bh_sb[:, 0:1])

                    # 32x32 block transpose into the batch output tile:
                    # outb[p, n0+32k+c] = hb[c, n0+32k+p]
                    nc.vector.transpose(outb[:, n0:n0 + NT], hb[:])

                    # reg columns += coarse residuals
                    treg = outb[:, n0:n0 + NT].rearrange(
                        "p (k c) -> p k c", c=32
                    )[:, :, NCLS:25]
                    ccv = ccb[:, t * CCW:(t + 1) * CCW].rearrange(
                        "p (k c) -> p k c", c=NREG
                    )
                    nc.vector.tensor_add(treg, treg, ccv)

                nc.scalar.dma_start(out_o[b], outb[:])
    if os.environ.get("BRB_SKIP_SPLIT") != "1":
        _split_multi_waits(nc)
    return nc


_NC_CACHE = {}


def _get_program():
    key = str(_MM_DT)
    if key not in _NC_CACHE:
        _NC_CACHE[key] = _build_program()
    return _NC_CACHE[key]


def _host_fold(inputs):
    """Fold BN into conv weights/biases; build per-core input maps."""
    f = lambda k: np.asarray(inputs[k], dtype=np.float32)
    W0, b0, g0, be0, m0, v0 = (f(k) for k in ("W0", "b0", "g0", "be0", "m0", "v0"))
    W1, b1, g1, be1, m1, v1 = (f(k) for k in ("W1", "b1", "g1", "be1", "m1", "v1"))
    Wc, bc, Wr, br = (f(k) for k in ("Wc", "bc", "Wr", "br"))

    s0 = g0 / np.sqrt(v0 + BN_EPS)
    W0f = W0 * s0[:, None]
    b0f = (b0 - m0) * s0 + be0
    s1 = g1 / np.sqrt(v1 + BN_EPS)
    W1f = W1 * s1[:, None]
    b1f = (b1 - m1) * s1 + be1

    w0T = np.ascontiguousarray(W0f.T)            # [256, 128]
    wh = np.zeros((C1, CH), np.float32)          # [128, 32]
    wh[:, 0:NCLS] = Wc.T
    wh[:, NCLS:25] = Wr.T
    bh = np.zeros((CH, 1), np.float32)
    bh[0:NCLS, 0] = bc
    bh[NCLS:25, 0] = br

    fused = np.asarray(inputs["fused_feats"], dtype=np.float32)
    # [B, 256, N] -> [B, 2, 128, N] K-chunk split (a reshape, no copy)
    fused = fused.reshape(B, 2, 128, N)

    angle = np.asarray(inputs["coarse_angle"], dtype=np.float32)
    dist = np.asarray(inputs["coarse_distance"], dtype=np.float32)
    cc = np.empty((B, N, NREG), np.float32)
    cc[:, :, 0] = angle
    cc[:, :, 1:] = dist
    # pre-arrange to the post-transpose SBUF layout:
    # cc_dev[b, p, (t*16 + k)*7 + c] = cc[b, t*512 + k*32 + p, c]
    cc_dev = np.ascontiguousarray(
        cc.reshape(B, NTILES, NT // 32, 32, NREG).transpose(0, 3, 1, 2, 4)
    ).reshape(B, 32, NTILES * (NT // 32) * NREG)

    shared = {
        "w0a": np.ascontiguousarray(w0T[0:128]),
        "w0b": np.ascontiguousarray(w0T[128:256]),
        "w1": np.ascontiguousarray(W1f.T),
        "wh": wh,
        "b0": b0f.reshape(C1, 1),
        "b1": b1f.reshape(C1, 1),
        "bh": bh,
    }
    in_maps = []
    for c in range(N_CORES):
        lo, hi = c * BPC, (c + 1) * BPC
        m = dict(shared)
        m["x"] = np.ascontiguousarray(fused[lo:hi])
        m["cc"] = np.ascontiguousarray(cc_dev[lo:hi])
        in_maps.append(m)
    return in_maps


def _run(inputs, trace=False):
    nc = _get_program()
    in_maps = _host_fold(inputs)
    res = run_bass_kernel_spmd(
        nc, in_maps, core_ids=list(range(N_CORES)), trace=trace
    )
    raw = np.concatenate([res.results[c]["out"] for c in range(N_CORES)], axis=0)
    # raw[b, p, t*512 + 32k + c] = channel c at position t*512 + k*32 + p
    full = np.ascontiguousarray(
        raw.reshape(B, 32, NTILES, NT // 32, 32).transpose(0, 2, 3, 1, 4)
    ).reshape(B, N, 32)
    sem_scores = np.ascontiguousarray(full[:, :, 0:NCLS])
    refined_angle = np.ascontiguousarray(full[:, :, NCLS])
    refined_distance = np.ascontiguousarray(full[:, :, NCLS + 1:25])
    return (sem_scores, refined_angle, refined_distance), res


def kernel(**inputs):
    outs, _ = _run(inputs, trace=False)
    return outs
